# revision 17
# baseline (speedup 1.0000x reference)
"""Multi-head causal attention (B=4, S=2048, D=1024, H=16) on 8 TRN2 NeuronCores.

Sharding: batch x head-group. Core c handles batch c//2 and heads
8*(c%2) .. 8*(c%2)+8 (tensor parallel over heads). Each core computes its
8 heads' attention plus a partial output projection; the host sums the two
fp16 partials per batch and adds b_out.

Single woven emission stream (v2): attention blocks run qt-major with
pairs sequential; all projection chains (qk, v, out-proj) are interleaved
into the attention stream so the PE consumes its slack under the
ScalarE(exp)-bound softmax instead of running as serial phases.
Diagonal-block matmuls stream only the live [zlo:] columns. PSUM budget:
psA(2) + psS(4) + psO(2) banks.
"""
import numpy as np

import concourse.bass as bass
import concourse.tile as tile
from concourse import bacc, mybir
from concourse import bass_utils

B, S, D, H, HD = 4, 2048, 1024, 16, 64
NCORES = 8
HPC = H // 2          # heads per core (8)
NPAIR = HPC // 2      # head pairs per core (4)
DC = HPC * HD         # attn dims per core (512)
QT = 512              # q tile (free dim of S^T)
KT = 128              # k tile (partition dim of S^T)
NQT = S // QT         # 4
NKT = S // KT         # 16
NTT = S // 128        # 16 token tiles
NCH = D // 128        # 8 d_model chunks
SCALE = HD ** -0.5
LAG = 5               # sexp -> av pipeline depth (blocks)

F32 = mybir.dt.float32
F16 = mybir.dt.float16

_cache = {}


def _classify_mask(mask):
    """Per (kt, qt) block: 0=skip (all masked), 1=full (none masked), 2=partial."""
    mask = np.asarray(mask).astype(bool)
    classes = np.zeros((NKT, NQT), np.int8)
    patterns = []
    pat_idx = {}
    pat_key = {}
    bounds = {}
    for qt in range(NQT):
        mb = mask[qt * QT:(qt + 1) * QT, :]          # [512, S] (q, k)
        for kt in range(NKT):
            blk = mb[:, kt * KT:(kt + 1) * KT]       # [512, 128] (q, k)
            if blk.all():
                classes[kt, qt] = 0
            elif not blk.any():
                classes[kt, qt] = 1
            else:
                classes[kt, qt] = 2
                tilev = (~blk).T.astype(np.float32)  # [128, 512] (k, q), 1=keep
                col_masked = tilev.min(axis=0) == 0.0
                col_dead = tilev.max(axis=0) == 0.0
                zlo = 0
                while zlo < QT and col_dead[zlo]:
                    zlo += 1
                nz = np.nonzero(col_masked[zlo:])[0]
                if len(nz):
                    mlo, mhi = zlo + int(nz[0]), zlo + int(nz[-1]) + 1
                else:
                    mlo, mhi = zlo, zlo
                key = tilev.tobytes()
                if key not in pat_key:
                    pat_key[key] = len(patterns)
                    patterns.append(tilev)
                pat_idx[(kt, qt)] = pat_key[key]
                bounds[(kt, qt)] = (zlo, mlo, mhi)
    if not patterns:
        patterns.append(np.ones((KT, QT), np.float32))
    return classes, np.stack(patterns), pat_idx, bounds


def _build(classes, pat_idx, bounds, n_pat):
    nc = bacc.Bacc("TRN2", target_bir_lowering=False, debug=False,
                   num_devices=NCORES)

    xT_d = nc.dram_tensor("xT", [D, S], F16, kind="ExternalInput")
    wqk_d = nc.dram_tensor("wqk", [D, 2 * DC], F16, kind="ExternalInput")
    wv_d = nc.dram_tensor("wv", [D, DC], F16, kind="ExternalInput")
    wo_d = nc.dram_tensor("wo", [DC, D], F16, kind="ExternalInput")
    bqk_d = nc.dram_tensor("bqk", [128, 2 * NPAIR], F32, kind="ExternalInput")
    bv_d = nc.dram_tensor("bv", [1, DC], F32, kind="ExternalInput")
    mpat_d = nc.dram_tensor("mpat", [n_pat, KT, QT], F16, kind="ExternalInput")
    out01_d = nc.dram_tensor("out01", [S, D], F16, kind="ExternalOutput")

    wqk_c = wqk_d.ap().rearrange("(c p) n -> c p n", p=128)
    wv_c = wv_d.ap().rearrange("(c p) n -> c p n", p=128)
    wo_c = wo_d.ap().rearrange("(c p) n -> c p n", p=128)
    xT_c = xT_d.ap().rearrange("(c p) n -> c p n", p=128)

    with tile.TileContext(nc) as tc:
        with (
            tc.tile_pool(name="persist", bufs=1) as persist,
            tc.tile_pool(name="bigpool", bufs=1) as bigpool,
        ):
            # ---- persistent tiles + all input DMAs --------------------
            qkT = [bigpool.tile([128, S], F16, name=f"qkT{p}", tag="qk",
                                bufs=8) for p in range(2 * NPAIR)]
            vext = [persist.tile([128, HPC, HD + 1], F16, name=f"vx{t}",
                                 tag=f"vx{t}") for t in range(NTT)]
            mpat = [persist.tile([KT, QT], F16, name=f"mp{i}", tag=f"mp{i}")
                    for i in range(n_pat)]
            outTn = [bigpool.tile([128, S], F16, name=f"oTn{p}", tag="big",
                                  bufs=4) for p in range(NPAIR)]
            bqk_sb = persist.tile([128, 2 * NPAIR], F32)
            bv_bc = persist.tile([128, DC], F32)

            # resident inputs: weights first, then x sliced by tt so the
            # first chains unblock after ~3MB of DMA.
            # Each dma_start trigger costs ~600ns on the issuing engine's
            # sequencer; rotate triggers across the four idle sequencers so
            # the head DMAs issue in parallel instead of serializing on
            # sync. wqk+x(tt0) interleaved ch-major: the first qk chain's
            # per-ch deps arrive in consumption order.
            dma_eng = [nc.sync, nc.scalar, nc.gpsimd]

            def dma_in(i, dst, src):
                dma_eng[i % 3].dma_start(dst, src)

            xT = [persist.tile([128, S], F16, name=f"x{ch}", tag=f"x{ch}")
                  for ch in range(NCH)]
            wqk_sb = [persist.tile([128, 2 * DC], F16, name=f"wqk{ch}",
                                   tag=f"wqk{ch}") for ch in range(NCH)]
            for ch in range(NCH):
                dma_in(2 * ch, wqk_sb[ch], wqk_c[ch])
                dma_in(2 * ch + 1, xT[ch][:, 0:QT], xT_c[ch][:, 0:QT])
            dma_in(0, bqk_sb, bqk_d.ap())
            dma_in(1, bv_bc,
                   bass.AP(tensor=bv_d, offset=0, ap=[[0, 128], [1, DC]]))
            for i in range(n_pat):
                dma_in(2 + i, mpat[i], mpat_d.ap()[i])
            wv_sb = [persist.tile([128, DC], F16, name=f"wv{ch}",
                                  tag=f"wv{ch}") for ch in range(NCH)]
            for ch in range(NCH):
                dma_in(ch, wv_sb[ch], wv_c[ch])
            # later-section inputs: trigger on sync only (not needed before
            # ~30us; keeps the compute engines' FIFOs clear)
            for tt in range(1, NQT):
                for ch in range(NCH):
                    nc.sync.dma_start(xT[ch][:, tt * QT:(tt + 1) * QT],
                                      xT_c[ch][:, tt * QT:(tt + 1) * QT])
            wo_sb = [persist.tile([128, D], F16, name=f"wo{p}", tag=f"wo{p}")
                     for p in range(NPAIR)]
            for p in range(NPAIR):
                nc.sync.dma_start(wo_sb[p], wo_c[p])

            with (
                tc.tile_pool(name="ppool", bufs=8) as ppool,
                tc.tile_pool(name="spool", bufs=4) as spool,
                tc.tile_pool(name="ocopy", bufs=6) as ocopy,
                tc.tile_pool(name="psA", bufs=2, space="PSUM") as psA,
                tc.tile_pool(name="psS", bufs=2, space="PSUM") as psS,
                tc.tile_pool(name="psO", bufs=1, space="PSUM") as psO,
            ):
                # ---- projection chain units ---------------------------
                def unit_qk(pp, tt):
                    def emit():
                        ps = psA.tile([128, QT], F32, tag="pa",
                                      name=f"psqk{pp}_{tt}")
                        for ch in range(NCH):
                            nc.tensor.matmul(
                                ps, wqk_sb[ch][:, pp * 128:(pp + 1) * 128],
                                xT[ch][:, tt * QT:(tt + 1) * QT],
                                start=(ch == 0), stop=(ch == NCH - 1))
                        nc.vector.tensor_scalar_add(
                            qkT[pp][:, tt * QT:(tt + 1) * QT], ps,
                            bqk_sb[:, pp:pp + 1])
                    return emit

                def unit_v(tt):
                    def emit():
                        ps = psA.tile([128, DC], F32, tag="pa",
                                      name=f"psv{tt}")
                        for ch in range(NCH):
                            nc.tensor.matmul(
                                ps, xT[ch][:, tt * 128:(tt + 1) * 128],
                                wv_sb[ch],
                                start=(ch == 0), stop=(ch == NCH - 1))
                        src3 = ps.rearrange("p (h d) -> p h d", h=HPC)
                        bv3 = bv_bc.rearrange("p (h d) -> p h d", h=HPC)
                        nc.vector.tensor_add(vext[tt][:, :, 0:HD], src3, bv3)
                        nc.vector.memset(vext[tt][:, :, HD:HD + 1], 1.0)
                    return emit

                def unit_d(tt, nt, tail=False):
                    def emit():
                        pso = psA.tile([128, QT], F32, tag="pa",
                                       name=f"pso{tt}_{nt}")
                        for p in range(NPAIR):
                            nc.tensor.matmul(
                                pso, outTn[p][:, tt * 128:(tt + 1) * 128],
                                wo_sb[p][:, nt * QT:(nt + 1) * QT],
                                start=(p == 0), stop=(p == NPAIR - 1))
                        ot = ocopy.tile([128, QT], F16, tag="oc")
                        if (tt + nt) % 2 == 1:
                            nc.scalar.activation(
                                ot, pso, mybir.ActivationFunctionType.Copy)
                        else:
                            nc.vector.tensor_copy(ot, pso)
                        dst = out01_d.ap()[tt * 128:(tt + 1) * 128,
                                           nt * QT:(nt + 1) * QT]
                        # tail DMAs gate the kernel end: issue their
                        # triggers from different sequencers in parallel
                        eng = dma_eng[(2 * tt + nt) % 3] if tail else nc.sync
                        eng.dma_start(dst, ot)
                    return emit

                # ---- attention block emitters -------------------------
                oT_live = {}
                pAB_live = {}

                def emit_sexp(ev):
                    p, qt, kt, first, last = ev
                    qTp, kTp = qkT[p], qkT[NPAIR + p]
                    if first:
                        oT_live[(p, qt)] = [
                            psO.tile([HD + 1, QT], F32,
                                     name=f"o{p}_{qt}_{h}", tag=f"o{h}")
                            for h in range(2)]
                    zlo, mlo, mhi = (0, 0, 0) if classes[kt, qt] == 1 \
                        else bounds[(kt, qt)]
                    sAB = psS.tile([128, 2, QT], F32, tag="sAB",
                                   name=f"s{p}_{qt}_{kt}")
                    for h in range(2):
                        nc.tensor.matmul(
                            sAB[:, h, zlo:],
                            kTp[64 * h:64 * h + 64, kt * KT:(kt + 1) * KT],
                            qTp[64 * h:64 * h + 64,
                                qt * QT + zlo:(qt + 1) * QT],
                            tile_position=(64 * h, 0))
                    pAB = ppool.tile([128, 2, QT], F16, tag="pAB",
                                     name=f"p{p}_{qt}_{kt}")
                    nc.scalar.activation(
                        pAB[:, :, zlo:], sAB[:, :, zlo:],
                        mybir.ActivationFunctionType.Exp, scale=SCALE)
                    if mhi > mlo:
                        pat = mpat[pat_idx[(kt, qt)]]
                        for h in range(2):
                            nc.vector.tensor_mul(
                                pAB[:, h, mlo:mhi], pAB[:, h, mlo:mhi],
                                pat[:, mlo:mhi])
                    pAB_live[(p, qt, kt)] = (pAB, zlo)

                def emit_av(ev):
                    p, qt, kt, first, last = ev
                    oT = oT_live[(p, qt)]
                    pAB, zlo = pAB_live.pop((p, qt, kt))
                    for h in range(2):
                        nc.tensor.matmul(
                            oT[h][:, zlo:], vext[kt][:, 2 * p + h, :],
                            pAB[:, h, zlo:], start=first, stop=last)
                    if last:
                        for h in range(2):
                            den = spool.tile([1, QT], F32, tag="den",
                                             name=f"d{p}_{qt}_{h}")
                            nc.vector.tensor_copy(den, oT[h][HD:HD + 1, :])
                            rec = spool.tile([1, QT], F32, tag="rec",
                                             name=f"r{p}_{qt}_{h}")
                            nc.vector.reciprocal_approx_fast(
                                out=rec, in_=den)
                            bc = spool.tile([HD, QT], F32, tag="bc",
                                            name=f"b{p}_{qt}_{h}")
                            nc.gpsimd.partition_broadcast(bc, rec[0:1, :])
                            nc.vector.tensor_mul(
                                outTn[p][64 * h:64 * h + 64,
                                         qt * QT:(qt + 1) * QT],
                                oT[h][0:HD, :], bc)

                # ---- global schedule ----------------------------------
                # attention blocks qt-major, pairs sequential within qt
                blocks = []
                sec_of = []
                for qt in range(NQT):
                    for p in range(NPAIR):
                        kts = [kt for kt in range(NKT)
                               if classes[kt, qt] != 0]
                        for i, kt in enumerate(kts):
                            blocks.append(
                                (p, qt, kt, i == 0, i == len(kts) - 1))
                            sec_of.append(qt)

                # units woven into each section (emitted during section s,
                # ahead of any consumer in section s+1)
                pre_units = [unit_qk(pp, 0) for pp in
                             (0, 4, 1, 5, 2, 6, 3, 7)] + \
                            [unit_v(t) for t in range(4)]
                weave = {qt: [] for qt in range(NQT)}
                weave[0] = [unit_qk(pp, 1) for pp in
                            (0, 4, 1, 5, 2, 6, 3, 7)] + \
                           [unit_v(t) for t in range(4, 8)]
                weave[1] = [unit_qk(pp, 2) for pp in
                            (0, 4, 1, 5, 2, 6, 3, 7)] + \
                           [unit_v(t) for t in range(8, 16)]
                weave[2] = [unit_qk(pp, 3) for pp in
                            (0, 4, 1, 5, 2, 6, 3, 7)] + \
                           [unit_d(tt, nt) for tt in range(0, 4)
                            for nt in range(2)]
                weave[3] = [unit_d(tt, nt) for tt in range(4, 12)
                            for nt in range(2)]
                post_units = [unit_d(tt, nt, tail=True)
                              for tt in range(12, 16) for nt in range(2)]

                # warm the PE (HAM un-throttles after ~3.4us of activity)
                # while the head DMAs stream in; results are discarded.
                warm = persist.tile([128, QT], F16, name="warm")
                nc.vector.memset(warm, 0.0)
                for w in range(16):
                    pw = psA.tile([64, QT], F32, tag="pa", name=f"warm{w}")
                    nc.tensor.matmul(pw, warm[:, 0:64], warm)

                for u in pre_units:
                    u()

                sec_start = {}
                sec_len = {}
                for i, s in enumerate(sec_of):
                    sec_start.setdefault(s, i)
                    sec_len[s] = sec_len.get(s, 0) + 1

                emitted = {qt: 0 for qt in range(NQT)}

                def pop_unit(s):
                    us = weave[s]
                    if emitted[s] < len(us):
                        us[emitted[s]]()
                        emitted[s] += 1
                        return True
                    return False

                for i in range(len(blocks) + LAG):
                    # av first: the score LDWEIGHTS of block i then load
                    # under the attnV streams instead of serializing
                    j = i - LAG
                    if j >= 0:
                        emit_av(blocks[j])
                        if blocks[j][4] and i < len(blocks):
                            # pair finished: give the PE chains to chew on
                            # while the normalize frees the psO bank
                            pop_unit(sec_of[i])
                            pop_unit(sec_of[i])
                    if i < len(blocks):
                        emit_sexp(blocks[i])
                        s = sec_of[i]
                        target = ((i - sec_start[s] + 1) * len(weave[s])
                                  + sec_len[s] - 1) // sec_len[s]
                        while emitted[s] < min(target, len(weave[s])):
                            if not pop_unit(s):
                                break

                for u in post_units:
                    u()

    nc.compile()
    return nc


def _prepare_inputs(x, mask, w_qkv, b_qkv, w_out):
    classes, patterns, pat_idx, bounds = _classify_mask(np.asarray(mask))
    in_maps = []
    for c in range(NCORES):
        b, g = c // 2, c % 2
        h0 = g * HPC
        xT = np.ascontiguousarray(x[b].T.astype(np.float16))
        wq = w_qkv[:, h0 * HD:h0 * HD + DC]
        wk = w_qkv[:, D + h0 * HD:D + h0 * HD + DC]
        wv = w_qkv[:, 2 * D + h0 * HD:2 * D + h0 * HD + DC]
        bq = b_qkv[h0 * HD:h0 * HD + DC]
        bk = b_qkv[D + h0 * HD:D + h0 * HD + DC]
        bv = b_qkv[2 * D + h0 * HD:2 * D + h0 * HD + DC]
        wo = w_out[h0 * HD:h0 * HD + DC, :]
        in_maps.append({
            "xT": xT,
            "wqk": np.ascontiguousarray(
                np.concatenate([wq, wk], axis=1).astype(np.float16)),
            "wv": np.ascontiguousarray(wv.astype(np.float16)),
            "wo": np.ascontiguousarray(wo.astype(np.float16)),
            "bqk": np.ascontiguousarray(
                np.concatenate([bq, bk]).reshape(2 * NPAIR, 128).T
                .astype(np.float32)),
            "bv": np.ascontiguousarray(bv[None, :].astype(np.float32)),
            "mpat": patterns.astype(np.float16),
        })
    return classes, patterns, pat_idx, bounds, in_maps


def run(x, mask, w_qkv, b_qkv, w_out, b_out, trace=False):
    classes, patterns, pat_idx, bounds, in_maps = _prepare_inputs(
        x, mask, w_qkv, b_qkv, w_out)
    key = (classes.tobytes(), patterns.tobytes())
    if key not in _cache:
        _cache[key] = _build(classes, pat_idx, bounds, patterns.shape[0])
    nc = _cache[key]
    res = bass_utils.run_bass_kernel_spmd(
        nc, in_maps, core_ids=list(range(NCORES)), trace=trace)
    out = np.empty((B, S, D), np.float32)
    bo = np.asarray(b_out, np.float32)
    for b in range(B):
        out[b] = (res.results[2 * b]["out01"].astype(np.float32)
                  + res.results[2 * b + 1]["out01"].astype(np.float32) + bo)
    return out, res


def kernel(x, mask, w_qkv, b_qkv, w_out, b_out):
    out, _ = run(x, mask, w_qkv, b_qkv, w_out, b_out, trace=False)
    return out


# revision 18
# speedup vs baseline: 1.0294x; 1.0294x over previous
"""Multi-head causal attention (B=4, S=2048, D=1024, H=16) on 8 TRN2 NeuronCores.

Sharding: batch x head-group. Core c handles batch c//2 and heads
8*(c%2) .. 8*(c%2)+8 (tensor parallel over heads). Each core computes its
8 heads' attention plus a partial output projection; the host sums the two
fp16 partials per batch and adds b_out.

Single woven emission stream (v2): attention blocks run qt-major with
pairs sequential; all projection chains (qk, v, out-proj) are interleaved
into the attention stream so the PE consumes its slack under the
ScalarE(exp)-bound softmax instead of running as serial phases.
Diagonal-block matmuls stream only the live [zlo:] columns. PSUM budget:
psA(2) + psS(4) + psO(2) banks.
"""
import numpy as np

import concourse.bass as bass
import concourse.tile as tile
from concourse import bacc, mybir
from concourse import bass_utils

B, S, D, H, HD = 4, 2048, 1024, 16, 64
NCORES = 8
HPC = H // 2          # heads per core (8)
NPAIR = HPC // 2      # head pairs per core (4)
DC = HPC * HD         # attn dims per core (512)
QT = 512              # q tile (free dim of S^T)
KT = 128              # k tile (partition dim of S^T)
NQT = S // QT         # 4
NKT = S // KT         # 16
NTT = S // 128        # 16 token tiles
NCH = D // 128        # 8 d_model chunks
SCALE = HD ** -0.5
LAG = 5               # sexp -> av pipeline depth (blocks)

F32 = mybir.dt.float32
F16 = mybir.dt.float16

_cache = {}


def _classify_mask(mask):
    """Per (kt, qt) block: 0=skip (all masked), 1=full (none masked), 2=partial."""
    mask = np.asarray(mask).astype(bool)
    classes = np.zeros((NKT, NQT), np.int8)
    patterns = []
    pat_idx = {}
    pat_key = {}
    bounds = {}
    for qt in range(NQT):
        mb = mask[qt * QT:(qt + 1) * QT, :]          # [512, S] (q, k)
        for kt in range(NKT):
            blk = mb[:, kt * KT:(kt + 1) * KT]       # [512, 128] (q, k)
            if blk.all():
                classes[kt, qt] = 0
            elif not blk.any():
                classes[kt, qt] = 1
            else:
                classes[kt, qt] = 2
                tilev = (~blk).T.astype(np.float32)  # [128, 512] (k, q), 1=keep
                col_masked = tilev.min(axis=0) == 0.0
                col_dead = tilev.max(axis=0) == 0.0
                zlo = 0
                while zlo < QT and col_dead[zlo]:
                    zlo += 1
                nz = np.nonzero(col_masked[zlo:])[0]
                if len(nz):
                    mlo, mhi = zlo + int(nz[0]), zlo + int(nz[-1]) + 1
                else:
                    mlo, mhi = zlo, zlo
                key = tilev.tobytes()
                if key not in pat_key:
                    pat_key[key] = len(patterns)
                    patterns.append(tilev)
                pat_idx[(kt, qt)] = pat_key[key]
                bounds[(kt, qt)] = (zlo, mlo, mhi)
    if not patterns:
        patterns.append(np.ones((KT, QT), np.float32))
    return classes, np.stack(patterns), pat_idx, bounds


def _build(classes, pat_idx, bounds, n_pat):
    nc = bacc.Bacc("TRN2", target_bir_lowering=False, debug=False,
                   num_devices=NCORES)

    xT_d = nc.dram_tensor("xT", [D, S], F16, kind="ExternalInput")
    wqk_d = nc.dram_tensor("wqk", [D, 2 * DC], F16, kind="ExternalInput")
    wv_d = nc.dram_tensor("wv", [D, DC], F16, kind="ExternalInput")
    wo_d = nc.dram_tensor("wo", [DC, D], F16, kind="ExternalInput")
    bqk_d = nc.dram_tensor("bqk", [128, 2 * NPAIR], F32, kind="ExternalInput")
    bv_d = nc.dram_tensor("bv", [1, DC], F32, kind="ExternalInput")
    mpat_d = nc.dram_tensor("mpat", [n_pat, KT, QT], F16, kind="ExternalInput")
    out01_d = nc.dram_tensor("out01", [S, D], F16, kind="ExternalOutput")

    wqk_c = wqk_d.ap().rearrange("(c p) n -> c p n", p=128)
    wv_c = wv_d.ap().rearrange("(c p) n -> c p n", p=128)
    wo_c = wo_d.ap().rearrange("(c p) n -> c p n", p=128)
    xT_c = xT_d.ap().rearrange("(c p) n -> c p n", p=128)

    with tile.TileContext(nc) as tc:
        with (
            tc.tile_pool(name="persist", bufs=1) as persist,
            tc.tile_pool(name="bigpool", bufs=1) as bigpool,
        ):
            # ---- persistent tiles + all input DMAs --------------------
            qkT = [bigpool.tile([128, S], F16, name=f"qkT{p}", tag="qk",
                                bufs=8) for p in range(2 * NPAIR)]
            vext = [persist.tile([128, HPC, HD + 1], F16, name=f"vx{t}",
                                 tag=f"vx{t}") for t in range(NTT)]
            mpat = [persist.tile([KT, QT], F16, name=f"mp{i}", tag=f"mp{i}")
                    for i in range(n_pat)]
            outTn = [bigpool.tile([128, S], F16, name=f"oTn{p}", tag="big",
                                  bufs=4) for p in range(NPAIR)]
            bqk_sb = persist.tile([128, 2 * NPAIR], F32)
            bv_bc = persist.tile([128, DC], F32)

            # resident inputs: weights first, then x sliced by tt so the
            # first chains unblock after ~3MB of DMA.
            # Each dma_start trigger costs ~600ns on the issuing engine's
            # sequencer; rotate triggers across the four idle sequencers so
            # the head DMAs issue in parallel instead of serializing on
            # sync. wqk+x(tt0) interleaved ch-major: the first qk chain's
            # per-ch deps arrive in consumption order.
            dma_eng = [nc.sync, nc.scalar, nc.gpsimd]

            def dma_in(i, dst, src):
                dma_eng[i % 3].dma_start(dst, src)

            xT = [persist.tile([128, S], F16, name=f"x{ch}", tag=f"x{ch}")
                  for ch in range(NCH)]
            wqk_sb = [persist.tile([128, 2 * DC], F16, name=f"wqk{ch}",
                                   tag=f"wqk{ch}") for ch in range(NCH)]
            for ch in range(NCH):
                dma_in(2 * ch, wqk_sb[ch], wqk_c[ch])
                dma_in(2 * ch + 1, xT[ch][:, 0:QT], xT_c[ch][:, 0:QT])
            dma_in(0, bqk_sb, bqk_d.ap())
            dma_in(1, bv_bc,
                   bass.AP(tensor=bv_d, offset=0, ap=[[0, 128], [1, DC]]))
            for i in range(n_pat):
                dma_in(2 + i, mpat[i], mpat_d.ap()[i])
            wv_sb = [persist.tile([128, DC], F16, name=f"wv{ch}",
                                  tag=f"wv{ch}") for ch in range(NCH)]
            for ch in range(NCH):
                dma_in(ch, wv_sb[ch], wv_c[ch])
            # later-section inputs: trigger on sync only (not needed before
            # ~30us; keeps the compute engines' FIFOs clear)
            for tt in range(1, NQT):
                for ch in range(NCH):
                    nc.sync.dma_start(xT[ch][:, tt * QT:(tt + 1) * QT],
                                      xT_c[ch][:, tt * QT:(tt + 1) * QT])
            wo_sb = [persist.tile([128, D], F16, name=f"wo{p}", tag=f"wo{p}")
                     for p in range(NPAIR)]
            for p in range(NPAIR):
                nc.sync.dma_start(wo_sb[p], wo_c[p])

            with (
                tc.tile_pool(name="ppool", bufs=8) as ppool,
                tc.tile_pool(name="spool", bufs=4) as spool,
                tc.tile_pool(name="ocopy", bufs=6) as ocopy,
                tc.tile_pool(name="psA", bufs=2, space="PSUM") as psA,
                tc.tile_pool(name="psS", bufs=2, space="PSUM") as psS,
                tc.tile_pool(name="psO", bufs=1, space="PSUM") as psO,
            ):
                # ---- projection chain units ---------------------------
                def unit_qk(pp, tt):
                    def emit():
                        ps = psA.tile([128, QT], F32, tag="pa",
                                      name=f"psqk{pp}_{tt}")
                        for ch in range(NCH):
                            nc.tensor.matmul(
                                ps, wqk_sb[ch][:, pp * 128:(pp + 1) * 128],
                                xT[ch][:, tt * QT:(tt + 1) * QT],
                                start=(ch == 0), stop=(ch == NCH - 1))
                        nc.vector.tensor_scalar_add(
                            qkT[pp][:, tt * QT:(tt + 1) * QT], ps,
                            bqk_sb[:, pp:pp + 1])
                    return emit

                def unit_v(tt):
                    def emit():
                        ps = psA.tile([128, DC], F32, tag="pa",
                                      name=f"psv{tt}")
                        for ch in range(NCH):
                            nc.tensor.matmul(
                                ps, xT[ch][:, tt * 128:(tt + 1) * 128],
                                wv_sb[ch],
                                start=(ch == 0), stop=(ch == NCH - 1))
                        src3 = ps.rearrange("p (h d) -> p h d", h=HPC)
                        bv3 = bv_bc.rearrange("p (h d) -> p h d", h=HPC)
                        nc.vector.tensor_add(vext[tt][:, :, 0:HD], src3, bv3)
                        nc.vector.memset(vext[tt][:, :, HD:HD + 1], 1.0)
                    return emit

                def unit_d(tt, nt, tail=False):
                    def emit():
                        pso = psA.tile([128, QT], F32, tag="pa",
                                       name=f"pso{tt}_{nt}")
                        for p in range(NPAIR):
                            nc.tensor.matmul(
                                pso, outTn[p][:, tt * 128:(tt + 1) * 128],
                                wo_sb[p][:, nt * QT:(nt + 1) * QT],
                                start=(p == 0), stop=(p == NPAIR - 1))
                        ot = ocopy.tile([128, QT], F16, tag="oc")
                        if tail and nt == 1:
                            nc.scalar.activation(
                                ot, pso, mybir.ActivationFunctionType.Copy)
                        else:
                            nc.vector.tensor_copy(ot, pso)
                        dst = out01_d.ap()[tt * 128:(tt + 1) * 128,
                                           nt * QT:(nt + 1) * QT]
                        # tail DMAs gate the kernel end: issue their
                        # triggers from different sequencers in parallel
                        eng = dma_eng[(2 * tt + nt) % 3] if tail else nc.sync
                        eng.dma_start(dst, ot)
                    return emit

                # ---- attention block emitters -------------------------
                oT_live = {}
                pAB_live = {}

                def emit_sexp(ev):
                    p, qt, kt, first, last = ev
                    qTp, kTp = qkT[p], qkT[NPAIR + p]
                    if first:
                        oT_live[(p, qt)] = [
                            psO.tile([HD + 1, QT], F32,
                                     name=f"o{p}_{qt}_{h}", tag=f"o{h}")
                            for h in range(2)]
                    zlo, mlo, mhi = (0, 0, 0) if classes[kt, qt] == 1 \
                        else bounds[(kt, qt)]
                    sAB = psS.tile([128, 2, QT], F32, tag="sAB",
                                   name=f"s{p}_{qt}_{kt}")
                    for h in range(2):
                        nc.tensor.matmul(
                            sAB[:, h, zlo:],
                            kTp[64 * h:64 * h + 64, kt * KT:(kt + 1) * KT],
                            qTp[64 * h:64 * h + 64,
                                qt * QT + zlo:(qt + 1) * QT],
                            tile_position=(64 * h, 0))
                    pAB = ppool.tile([128, 2, QT], F16, tag="pAB",
                                     name=f"p{p}_{qt}_{kt}")
                    nc.scalar.activation(
                        pAB[:, :, zlo:], sAB[:, :, zlo:],
                        mybir.ActivationFunctionType.Exp, scale=SCALE)
                    if mhi > mlo:
                        pat = mpat[pat_idx[(kt, qt)]]
                        for h in range(2):
                            nc.vector.tensor_mul(
                                pAB[:, h, mlo:mhi], pAB[:, h, mlo:mhi],
                                pat[:, mlo:mhi])
                    pAB_live[(p, qt, kt)] = (pAB, zlo)

                def emit_av(ev):
                    p, qt, kt, first, last = ev
                    oT = oT_live[(p, qt)]
                    pAB, zlo = pAB_live.pop((p, qt, kt))
                    for h in range(2):
                        nc.tensor.matmul(
                            oT[h][:, zlo:], vext[kt][:, 2 * p + h, :],
                            pAB[:, h, zlo:], start=first, stop=last)
                    if last:
                        for h in range(2):
                            den = spool.tile([1, QT], F32, tag="den",
                                             name=f"d{p}_{qt}_{h}")
                            nc.vector.tensor_copy(den, oT[h][HD:HD + 1, :])
                            rec = spool.tile([1, QT], F32, tag="rec",
                                             name=f"r{p}_{qt}_{h}")
                            nc.vector.reciprocal_approx_fast(
                                out=rec, in_=den)
                            bc = spool.tile([HD, QT], F32, tag="bc",
                                            name=f"b{p}_{qt}_{h}")
                            nc.gpsimd.partition_broadcast(bc, rec[0:1, :])
                            nc.vector.tensor_mul(
                                outTn[p][64 * h:64 * h + 64,
                                         qt * QT:(qt + 1) * QT],
                                oT[h][0:HD, :], bc)

                # ---- global schedule ----------------------------------
                # attention blocks qt-major, pairs sequential within qt
                blocks = []
                sec_of = []
                for qt in range(NQT):
                    for p in range(NPAIR):
                        kts = [kt for kt in range(NKT)
                               if classes[kt, qt] != 0]
                        for i, kt in enumerate(kts):
                            blocks.append(
                                (p, qt, kt, i == 0, i == len(kts) - 1))
                            sec_of.append(qt)

                # units woven into each section (emitted during section s,
                # ahead of any consumer in section s+1)
                pre_units = [unit_qk(pp, 0) for pp in
                             (0, 4, 1, 5, 2, 6, 3, 7)] + \
                            [unit_v(t) for t in range(4)]
                weave = {qt: [] for qt in range(NQT)}
                weave[0] = [unit_qk(pp, 1) for pp in
                            (0, 4, 1, 5, 2, 6, 3, 7)] + \
                           [unit_v(t) for t in range(4, 8)]
                weave[1] = [unit_qk(pp, 2) for pp in
                            (0, 4, 1, 5, 2, 6, 3, 7)] + \
                           [unit_v(t) for t in range(8, 16)]
                weave[2] = [unit_qk(pp, 3) for pp in
                            (0, 4, 1, 5, 2, 6, 3, 7)] + \
                           [unit_d(tt, nt) for tt in range(0, 4)
                            for nt in range(2)]
                weave[3] = [unit_d(tt, nt) for tt in range(4, 12)
                            for nt in range(2)]
                post_units = [unit_d(tt, nt, tail=True)
                              for tt in range(12, 16) for nt in range(2)]

                # warm the PE (HAM un-throttles after ~3.4us of activity)
                # while the head DMAs stream in; results are discarded.
                warm = persist.tile([128, QT], F16, name="warm")
                nc.vector.memset(warm, 0.0)
                for w in range(16):
                    pw = psA.tile([64, QT], F32, tag="pa", name=f"warm{w}")
                    nc.tensor.matmul(pw, warm[:, 0:64], warm)

                for u in pre_units:
                    u()

                sec_start = {}
                sec_len = {}
                for i, s in enumerate(sec_of):
                    sec_start.setdefault(s, i)
                    sec_len[s] = sec_len.get(s, 0) + 1

                emitted = {qt: 0 for qt in range(NQT)}

                def pop_unit(s):
                    us = weave[s]
                    if emitted[s] < len(us):
                        us[emitted[s]]()
                        emitted[s] += 1
                        return True
                    return False

                for i in range(len(blocks) + LAG):
                    # av first: the score LDWEIGHTS of block i then load
                    # under the attnV streams instead of serializing
                    j = i - LAG
                    if j >= 0:
                        emit_av(blocks[j])
                        if blocks[j][4] and i < len(blocks):
                            # pair finished: give the PE chains to chew on
                            # while the normalize frees the psO bank
                            pop_unit(sec_of[i])
                            pop_unit(sec_of[i])
                    if i < len(blocks):
                        emit_sexp(blocks[i])
                        s = sec_of[i]
                        target = ((i - sec_start[s] + 1) * len(weave[s])
                                  + sec_len[s] - 1) // sec_len[s]
                        while emitted[s] < min(target, len(weave[s])):
                            if not pop_unit(s):
                                break

                for u in post_units:
                    u()

    nc.compile()
    return nc


def _prepare_inputs(x, mask, w_qkv, b_qkv, w_out):
    classes, patterns, pat_idx, bounds = _classify_mask(np.asarray(mask))
    in_maps = []
    for c in range(NCORES):
        b, g = c // 2, c % 2
        h0 = g * HPC
        xT = np.ascontiguousarray(x[b].T.astype(np.float16))
        wq = w_qkv[:, h0 * HD:h0 * HD + DC]
        wk = w_qkv[:, D + h0 * HD:D + h0 * HD + DC]
        wv = w_qkv[:, 2 * D + h0 * HD:2 * D + h0 * HD + DC]
        bq = b_qkv[h0 * HD:h0 * HD + DC]
        bk = b_qkv[D + h0 * HD:D + h0 * HD + DC]
        bv = b_qkv[2 * D + h0 * HD:2 * D + h0 * HD + DC]
        wo = w_out[h0 * HD:h0 * HD + DC, :]
        in_maps.append({
            "xT": xT,
            "wqk": np.ascontiguousarray(
                np.concatenate([wq, wk], axis=1).astype(np.float16)),
            "wv": np.ascontiguousarray(wv.astype(np.float16)),
            "wo": np.ascontiguousarray(wo.astype(np.float16)),
            "bqk": np.ascontiguousarray(
                np.concatenate([bq, bk]).reshape(2 * NPAIR, 128).T
                .astype(np.float32)),
            "bv": np.ascontiguousarray(bv[None, :].astype(np.float32)),
            "mpat": patterns.astype(np.float16),
        })
    return classes, patterns, pat_idx, bounds, in_maps


def run(x, mask, w_qkv, b_qkv, w_out, b_out, trace=False):
    classes, patterns, pat_idx, bounds, in_maps = _prepare_inputs(
        x, mask, w_qkv, b_qkv, w_out)
    key = (classes.tobytes(), patterns.tobytes())
    if key not in _cache:
        _cache[key] = _build(classes, pat_idx, bounds, patterns.shape[0])
    nc = _cache[key]
    res = bass_utils.run_bass_kernel_spmd(
        nc, in_maps, core_ids=list(range(NCORES)), trace=trace)
    out = np.empty((B, S, D), np.float32)
    bo = np.asarray(b_out, np.float32)
    for b in range(B):
        out[b] = (res.results[2 * b]["out01"].astype(np.float32)
                  + res.results[2 * b + 1]["out01"].astype(np.float32) + bo)
    return out, res


def kernel(x, mask, w_qkv, b_qkv, w_out, b_out):
    out, _ = run(x, mask, w_qkv, b_qkv, w_out, b_out, trace=False)
    return out


# revision 21
# speedup vs baseline: 1.0421x; 1.0123x over previous
"""Multi-head causal attention (B=4, S=2048, D=1024, H=16) on 8 TRN2 NeuronCores.

Sharding: batch x head-group. Core c handles batch c//2 and heads
8*(c%2) .. 8*(c%2)+8 (tensor parallel over heads). Each core computes its
8 heads' attention plus a partial output projection; the host sums the two
fp16 partials per batch and adds b_out.

Single woven emission stream: attention blocks run qt-major with pairs
sequential; all projection chains (qk, v, out-proj) are interleaved into
the attention stream so the PE consumes its slack under the
ScalarE(exp)-bound softmax instead of running as serial phases.
Diagonal-block matmuls stream only the live [zlo:] columns (upper-left
dead columns skipped in scores, exp, and attn@V). attn@V pairs are
emitted before the next block's scores so score LDWEIGHTS overlap the
attn@V streams. PSUM budget: psA(2) + psS(4) + psO(2) banks = 8.

Startup: 16 discarded warmup matmuls un-throttle the PE HAM clock gate
while input DMAs stream; DMA triggers rotate across the sync/scalar/
gpsimd sequencers (each dma_start costs ~600ns on its issuing engine)
with first-needed tensors (wqk, x tt0) interleaved ch-major. Outputs are
fp16 partials (host sums in fp32 and adds b_out).
"""
import numpy as np

import concourse.bass as bass
import concourse.tile as tile
from concourse import bacc, mybir
from concourse import bass_utils

B, S, D, H, HD = 4, 2048, 1024, 16, 64
NCORES = 8
HPC = H // 2          # heads per core (8)
NPAIR = HPC // 2      # head pairs per core (4)
DC = HPC * HD         # attn dims per core (512)
QT = 512              # q tile (free dim of S^T)
KT = 128              # k tile (partition dim of S^T)
NQT = S // QT         # 4
NKT = S // KT         # 16
NTT = S // 128        # 16 token tiles
NCH = D // 128        # 8 d_model chunks
SCALE = HD ** -0.5
LAG = 4               # sexp -> av pipeline depth (blocks)

F32 = mybir.dt.float32
F16 = mybir.dt.float16

_cache = {}


def _classify_mask(mask):
    """Per (kt, qt) block: 0=skip (all masked), 1=full (none masked), 2=partial."""
    mask = np.asarray(mask).astype(bool)
    classes = np.zeros((NKT, NQT), np.int8)
    patterns = []
    pat_idx = {}
    pat_key = {}
    bounds = {}
    for qt in range(NQT):
        mb = mask[qt * QT:(qt + 1) * QT, :]          # [512, S] (q, k)
        for kt in range(NKT):
            blk = mb[:, kt * KT:(kt + 1) * KT]       # [512, 128] (q, k)
            if blk.all():
                classes[kt, qt] = 0
            elif not blk.any():
                classes[kt, qt] = 1
            else:
                classes[kt, qt] = 2
                tilev = (~blk).T.astype(np.float32)  # [128, 512] (k, q), 1=keep
                col_masked = tilev.min(axis=0) == 0.0
                col_dead = tilev.max(axis=0) == 0.0
                zlo = 0
                while zlo < QT and col_dead[zlo]:
                    zlo += 1
                nz = np.nonzero(col_masked[zlo:])[0]
                if len(nz):
                    mlo, mhi = zlo + int(nz[0]), zlo + int(nz[-1]) + 1
                else:
                    mlo, mhi = zlo, zlo
                key = tilev.tobytes()
                if key not in pat_key:
                    pat_key[key] = len(patterns)
                    patterns.append(tilev)
                pat_idx[(kt, qt)] = pat_key[key]
                bounds[(kt, qt)] = (zlo, mlo, mhi)
    if not patterns:
        patterns.append(np.ones((KT, QT), np.float32))
    return classes, np.stack(patterns), pat_idx, bounds


def _build(classes, pat_idx, bounds, n_pat):
    nc = bacc.Bacc("TRN2", target_bir_lowering=False, debug=False,
                   num_devices=NCORES)

    xT_d = nc.dram_tensor("xT", [D, S], F16, kind="ExternalInput")
    wqk_d = nc.dram_tensor("wqk", [D, 2 * DC], F16, kind="ExternalInput")
    wv_d = nc.dram_tensor("wv", [D, DC], F16, kind="ExternalInput")
    wo_d = nc.dram_tensor("wo", [DC, D], F16, kind="ExternalInput")
    bqk_d = nc.dram_tensor("bqk", [128, 2 * NPAIR], F32, kind="ExternalInput")
    bv_d = nc.dram_tensor("bv", [1, DC], F32, kind="ExternalInput")
    mpat_d = nc.dram_tensor("mpat", [n_pat, KT, QT], F16, kind="ExternalInput")
    out01_d = nc.dram_tensor("out01", [S, D], F16, kind="ExternalOutput")

    wqk_c = wqk_d.ap().rearrange("(c p) n -> c p n", p=128)
    wv_c = wv_d.ap().rearrange("(c p) n -> c p n", p=128)
    wo_c = wo_d.ap().rearrange("(c p) n -> c p n", p=128)
    xT_c = xT_d.ap().rearrange("(c p) n -> c p n", p=128)

    with tile.TileContext(nc) as tc:
        with (
            tc.tile_pool(name="persist", bufs=1) as persist,
            tc.tile_pool(name="bigpool", bufs=1) as bigpool,
        ):
            # ---- persistent tiles + all input DMAs --------------------
            qkT = [bigpool.tile([128, S], F16, name=f"qkT{p}", tag="qk",
                                bufs=8) for p in range(2 * NPAIR)]
            vext = [persist.tile([128, HPC, HD + 1], F16, name=f"vx{t}",
                                 tag=f"vx{t}") for t in range(NTT)]
            mpat = [persist.tile([KT, QT], F16, name=f"mp{i}", tag=f"mp{i}")
                    for i in range(n_pat)]
            outTn = [bigpool.tile([128, S], F16, name=f"oTn{p}", tag="big",
                                  bufs=4) for p in range(NPAIR)]
            bqk_sb = persist.tile([128, 2 * NPAIR], F32)
            bv_bc = persist.tile([128, DC], F32)

            # resident inputs: weights first, then x sliced by tt so the
            # first chains unblock after ~3MB of DMA.
            # Each dma_start trigger costs ~600ns on the issuing engine's
            # sequencer; rotate triggers across the four idle sequencers so
            # the head DMAs issue in parallel instead of serializing on
            # sync. wqk+x(tt0) interleaved ch-major: the first qk chain's
            # per-ch deps arrive in consumption order.
            dma_eng = [nc.sync, nc.scalar, nc.gpsimd]

            def dma_in(i, dst, src):
                dma_eng[i % 3].dma_start(dst, src)

            xT = [persist.tile([128, S], F16, name=f"x{ch}", tag=f"x{ch}")
                  for ch in range(NCH)]
            wqk_sb = [persist.tile([128, 2 * DC], F16, name=f"wqk{ch}",
                                   tag=f"wqk{ch}") for ch in range(NCH)]
            for ch in range(NCH):
                dma_in(2 * ch, wqk_sb[ch], wqk_c[ch])
                dma_in(2 * ch + 1, xT[ch][:, 0:QT], xT_c[ch][:, 0:QT])
            dma_in(0, bqk_sb, bqk_d.ap())
            dma_in(1, bv_bc,
                   bass.AP(tensor=bv_d, offset=0, ap=[[0, 128], [1, DC]]))
            for i in range(n_pat):
                dma_in(2 + i, mpat[i], mpat_d.ap()[i])
            wv_sb = [persist.tile([128, DC], F16, name=f"wv{ch}",
                                  tag=f"wv{ch}") for ch in range(NCH)]
            for ch in range(NCH):
                dma_in(ch, wv_sb[ch], wv_c[ch])
            # later-section inputs: trigger on sync only (not needed before
            # ~30us; keeps the compute engines' FIFOs clear)
            for tt in range(1, NQT):
                for ch in range(NCH):
                    nc.sync.dma_start(xT[ch][:, tt * QT:(tt + 1) * QT],
                                      xT_c[ch][:, tt * QT:(tt + 1) * QT])
            wo_sb = [persist.tile([128, D], F16, name=f"wo{p}", tag=f"wo{p}")
                     for p in range(NPAIR)]
            for p in range(NPAIR):
                nc.sync.dma_start(wo_sb[p], wo_c[p])

            with (
                tc.tile_pool(name="ppool", bufs=8) as ppool,
                tc.tile_pool(name="spool", bufs=4) as spool,
                tc.tile_pool(name="ocopy", bufs=6) as ocopy,
                tc.tile_pool(name="psA", bufs=2, space="PSUM") as psA,
                tc.tile_pool(name="psS", bufs=2, space="PSUM") as psS,
                tc.tile_pool(name="psO", bufs=1, space="PSUM") as psO,
            ):
                # ---- projection chain units ---------------------------
                def unit_qk(pp, tt):
                    def emit():
                        ps = psA.tile([128, QT], F32, tag="pa",
                                      name=f"psqk{pp}_{tt}")
                        for ch in range(NCH):
                            nc.tensor.matmul(
                                ps, wqk_sb[ch][:, pp * 128:(pp + 1) * 128],
                                xT[ch][:, tt * QT:(tt + 1) * QT],
                                start=(ch == 0), stop=(ch == NCH - 1))
                        nc.vector.tensor_scalar_add(
                            qkT[pp][:, tt * QT:(tt + 1) * QT], ps,
                            bqk_sb[:, pp:pp + 1])
                    return emit

                def unit_v(tt):
                    def emit():
                        ps = psA.tile([128, DC], F32, tag="pa",
                                      name=f"psv{tt}")
                        for ch in range(NCH):
                            nc.tensor.matmul(
                                ps, xT[ch][:, tt * 128:(tt + 1) * 128],
                                wv_sb[ch],
                                start=(ch == 0), stop=(ch == NCH - 1))
                        src3 = ps.rearrange("p (h d) -> p h d", h=HPC)
                        bv3 = bv_bc.rearrange("p (h d) -> p h d", h=HPC)
                        nc.vector.tensor_add(vext[tt][:, :, 0:HD], src3, bv3)
                        nc.vector.memset(vext[tt][:, :, HD:HD + 1], 1.0)
                    return emit

                def unit_d(tt, nt, tail=False):
                    def emit():
                        pso = psA.tile([128, QT], F32, tag="pa",
                                       name=f"pso{tt}_{nt}")
                        for p in range(NPAIR):
                            nc.tensor.matmul(
                                pso, outTn[p][:, tt * 128:(tt + 1) * 128],
                                wo_sb[p][:, nt * QT:(nt + 1) * QT],
                                start=(p == 0), stop=(p == NPAIR - 1))
                        ot = ocopy.tile([128, QT], F16, tag="oc")
                        if tail and nt == 1:
                            nc.scalar.activation(
                                ot, pso, mybir.ActivationFunctionType.Copy)
                        else:
                            nc.vector.tensor_copy(ot, pso)
                        dst = out01_d.ap()[tt * 128:(tt + 1) * 128,
                                           nt * QT:(nt + 1) * QT]
                        # tail DMAs gate the kernel end: issue their
                        # triggers from different sequencers in parallel
                        eng = dma_eng[(2 * tt + nt) % 3] if tail else nc.sync
                        eng.dma_start(dst, ot)
                    return emit

                # ---- attention block emitters -------------------------
                oT_live = {}
                pAB_live = {}

                def emit_sexp(ev):
                    p, qt, kt, first, last = ev
                    qTp, kTp = qkT[p], qkT[NPAIR + p]
                    if first:
                        oT_live[(p, qt)] = [
                            psO.tile([HD + 1, QT], F32,
                                     name=f"o{p}_{qt}_{h}", tag=f"o{h}")
                            for h in range(2)]
                    zlo, mlo, mhi = (0, 0, 0) if classes[kt, qt] == 1 \
                        else bounds[(kt, qt)]
                    sAB = psS.tile([128, 2, QT], F32, tag="sAB",
                                   name=f"s{p}_{qt}_{kt}")
                    for h in range(2):
                        nc.tensor.matmul(
                            sAB[:, h, zlo:],
                            kTp[64 * h:64 * h + 64, kt * KT:(kt + 1) * KT],
                            qTp[64 * h:64 * h + 64,
                                qt * QT + zlo:(qt + 1) * QT],
                            tile_position=(64 * h, 0))
                    pAB = ppool.tile([128, 2, QT], F16, tag="pAB",
                                     name=f"p{p}_{qt}_{kt}")
                    nc.scalar.activation(
                        pAB[:, :, zlo:], sAB[:, :, zlo:],
                        mybir.ActivationFunctionType.Exp, scale=SCALE)
                    if mhi > mlo:
                        pat = mpat[pat_idx[(kt, qt)]]
                        for h in range(2):
                            nc.vector.tensor_mul(
                                pAB[:, h, mlo:mhi], pAB[:, h, mlo:mhi],
                                pat[:, mlo:mhi])
                    pAB_live[(p, qt, kt)] = (pAB, zlo)

                def emit_av(ev):
                    p, qt, kt, first, last = ev
                    oT = oT_live[(p, qt)]
                    pAB, zlo = pAB_live.pop((p, qt, kt))
                    for h in range(2):
                        nc.tensor.matmul(
                            oT[h][:, zlo:], vext[kt][:, 2 * p + h, :],
                            pAB[:, h, zlo:], start=first, stop=last)
                    if last:
                        for h in range(2):
                            den = spool.tile([1, QT], F32, tag="den",
                                             name=f"d{p}_{qt}_{h}")
                            nc.vector.tensor_copy(den, oT[h][HD:HD + 1, :])
                            rec = spool.tile([1, QT], F32, tag="rec",
                                             name=f"r{p}_{qt}_{h}")
                            nc.vector.reciprocal_approx_fast(
                                out=rec, in_=den)
                            bc = spool.tile([HD, QT], F32, tag="bc",
                                            name=f"b{p}_{qt}_{h}")
                            nc.gpsimd.partition_broadcast(bc, rec[0:1, :])
                            nc.vector.tensor_mul(
                                outTn[p][64 * h:64 * h + 64,
                                         qt * QT:(qt + 1) * QT],
                                oT[h][0:HD, :], bc)

                # ---- global schedule ----------------------------------
                # attention blocks qt-major, pairs sequential within qt
                blocks = []
                sec_of = []
                for qt in range(NQT):
                    for p in range(NPAIR):
                        kts = [kt for kt in range(NKT)
                               if classes[kt, qt] != 0]
                        for i, kt in enumerate(kts):
                            blocks.append(
                                (p, qt, kt, i == 0, i == len(kts) - 1))
                            sec_of.append(qt)

                # units woven into each section (emitted during section s,
                # ahead of any consumer in section s+1)
                pre_units = [unit_qk(pp, 0) for pp in
                             (0, 4, 1, 5, 2, 6, 3, 7)] + \
                            [unit_v(t) for t in range(4)]
                weave = {qt: [] for qt in range(NQT)}
                weave[0] = [unit_qk(pp, 1) for pp in
                            (0, 4, 1, 5, 2, 6, 3, 7)] + \
                           [unit_v(t) for t in range(4, 8)]
                weave[1] = [unit_qk(pp, 2) for pp in
                            (0, 4, 1, 5, 2, 6, 3, 7)] + \
                           [unit_v(t) for t in range(8, 16)]
                weave[2] = [unit_qk(pp, 3) for pp in
                            (0, 4, 1, 5, 2, 6, 3, 7)] + \
                           [unit_d(tt, nt) for tt in range(0, 4)
                            for nt in range(2)]
                weave[3] = [unit_d(tt, nt) for tt in range(4, 12)
                            for nt in range(2)]
                post_units = [unit_d(tt, nt, tail=True)
                              for tt in range(12, 16) for nt in range(2)]

                # warm the PE (HAM un-throttles after ~3.4us of activity)
                # while the head DMAs stream in; results are discarded.
                warm = persist.tile([128, QT], F16, name="warm")
                nc.vector.memset(warm, 0.0)
                for w in range(16):
                    pw = psA.tile([64, QT], F32, tag="pa", name=f"warm{w}")
                    nc.tensor.matmul(pw, warm[:, 0:64], warm)

                for u in pre_units:
                    u()

                sec_start = {}
                sec_len = {}
                for i, s in enumerate(sec_of):
                    sec_start.setdefault(s, i)
                    sec_len[s] = sec_len.get(s, 0) + 1

                emitted = {qt: 0 for qt in range(NQT)}

                def pop_unit(s):
                    us = weave[s]
                    if emitted[s] < len(us):
                        us[emitted[s]]()
                        emitted[s] += 1
                        return True
                    return False

                for i in range(len(blocks) + LAG):
                    # av first: the score LDWEIGHTS of block i then load
                    # under the attnV streams instead of serializing
                    j = i - LAG
                    if j >= 0:
                        emit_av(blocks[j])
                        if blocks[j][4] and i < len(blocks):
                            # pair finished: give the PE a chain to chew on
                            # while the normalize frees the psO bank
                            pop_unit(sec_of[i])
                    if i < len(blocks):
                        emit_sexp(blocks[i])
                        s = sec_of[i]
                        target = ((i - sec_start[s] + 1) * len(weave[s])
                                  + sec_len[s] - 1) // sec_len[s]
                        while emitted[s] < min(target, len(weave[s])):
                            if not pop_unit(s):
                                break

                for u in post_units:
                    u()

    nc.compile()
    return nc


def _prepare_inputs(x, mask, w_qkv, b_qkv, w_out):
    classes, patterns, pat_idx, bounds = _classify_mask(np.asarray(mask))
    in_maps = []
    for c in range(NCORES):
        b, g = c // 2, c % 2
        h0 = g * HPC
        xT = np.ascontiguousarray(x[b].T.astype(np.float16))
        wq = w_qkv[:, h0 * HD:h0 * HD + DC]
        wk = w_qkv[:, D + h0 * HD:D + h0 * HD + DC]
        wv = w_qkv[:, 2 * D + h0 * HD:2 * D + h0 * HD + DC]
        bq = b_qkv[h0 * HD:h0 * HD + DC]
        bk = b_qkv[D + h0 * HD:D + h0 * HD + DC]
        bv = b_qkv[2 * D + h0 * HD:2 * D + h0 * HD + DC]
        wo = w_out[h0 * HD:h0 * HD + DC, :]
        in_maps.append({
            "xT": xT,
            "wqk": np.ascontiguousarray(
                np.concatenate([wq, wk], axis=1).astype(np.float16)),
            "wv": np.ascontiguousarray(wv.astype(np.float16)),
            "wo": np.ascontiguousarray(wo.astype(np.float16)),
            "bqk": np.ascontiguousarray(
                np.concatenate([bq, bk]).reshape(2 * NPAIR, 128).T
                .astype(np.float32)),
            "bv": np.ascontiguousarray(bv[None, :].astype(np.float32)),
            "mpat": patterns.astype(np.float16),
        })
    return classes, patterns, pat_idx, bounds, in_maps


def run(x, mask, w_qkv, b_qkv, w_out, b_out, trace=False):
    classes, patterns, pat_idx, bounds, in_maps = _prepare_inputs(
        x, mask, w_qkv, b_qkv, w_out)
    key = (classes.tobytes(), patterns.tobytes())
    if key not in _cache:
        _cache[key] = _build(classes, pat_idx, bounds, patterns.shape[0])
    nc = _cache[key]
    res = bass_utils.run_bass_kernel_spmd(
        nc, in_maps, core_ids=list(range(NCORES)), trace=trace)
    out = np.empty((B, S, D), np.float32)
    bo = np.asarray(b_out, np.float32)
    for b in range(B):
        out[b] = (res.results[2 * b]["out01"].astype(np.float32)
                  + res.results[2 * b + 1]["out01"].astype(np.float32) + bo)
    return out, res


def kernel(x, mask, w_qkv, b_qkv, w_out, b_out):
    out, _ = run(x, mask, w_qkv, b_qkv, w_out, b_out, trace=False)
    return out
